# revision 9
# baseline (speedup 1.0000x reference)
"""CantorSetAttention Trainium2 kernel (8 NeuronCores, data-parallel).

Reference computes, for depths d=0..7, attention of every query against the
tiny Cantor index set S_d (|S_d| = 2,3,5,9,17,33,65,129; sets are nested),
then blends the 8 outputs with w = softmax(scale_weights / scale_temperature).

Fusion used here:
  A[q,j] = sum_d w_d * 1[j in S_d] * E[q,j] / Z_d(q),  E = exp(q.k_j / sqrt(D))
  rows of A sum to exactly 1 (each softmax sums to 1, sum_d w_d = 1), so with
  j* = index 0 (member of every S_d):
     out[q] = sum_{j != j*} A[q,j] * (V[j] - V[j*])  +  V[j*]
  The union minus j* is exactly 128 indices -> fits the 128-partition PE.
  Keys are shifted host-side (K' = k_j - k_0) so the j* rank-1 term of each
  softmax denominator becomes the constant 1 (E~_{j*} = 1): no e* stream.

Engine budget (per core per rep, cost-model): PE 72 matmuls ~15.4us; the
PSUM->SBUF fp16 output copies (16 x [128,1024]) + exp/mul/recip/add split
across ACT/DVE at ~14-15us each; HWDGE DMA issue costs the issuing engine
~3ns/KB, so ALL q-input issues ride SP and ALL output drains ride the
GPSIMD SWDGE ring, keeping ACT/DVE free for compute.

Device pipeline: pair-level depth-2 software pipeline over 512-query block
pairs. Step p emits on PE: C(p-2) x2, then 8 chunk-interleaved ST(p) matmul
pairs with AV(p-2) halves woven between them (paces the PSUM drain), with
Z(p-1) matmuls dropped in at chunk 1 and 5 where their PSUM WARs are slack.
  ST[k,q] = K'_128 @ Q^T (f32 PSUM); E = exp(ST/32) (ACT)
  Z[8,q] = M^T E; +1 add (knob: ACT/DVE); R = 1/Z (DVE fp16)
  C = (w*M) R; A = E * C (DVE); P[q,:] = A^T (V - v*) fp16 AV matmuls.
Host adds v* back and upcasts to f32.
"""

import math
import os

import numpy as np

import concourse.bass as bass
import concourse.mybir as mybir
from concourse.bass_utils import run_bass_kernel_spmd
from concourse.tile import TileContext, add_dep_helper

B, L, D = 4, 4096, 1024
NCORES = 8
ROWS_PER_CORE = (B * L) // NCORES  # 2048
N_DEPTHS = 8
INV_SQRT_D = 1.0 / math.sqrt(D)
BLK = 512  # query block per ST/E/Z/C round
NBLK = ROWS_PER_CORE // BLK  # 4
NTIL = BLK // 128  # 4
F16 = mybir.dt.float16
F32 = mybir.dt.float32

_SMALL_COLS = 128  # m8w only

# knobs (read at build time; env for sim sweeps)
_KZ = os.environ.get("KZ", "act")  # "act" | "dve": engine for the z +1 add
# copy-engine per (block-in-pair, tile): 'A' = ACT, 'D' = DVE
_KCOPY = os.environ.get("KCOPY", "ADAD,ADAA")


def _cantor_indices(seq_len: int, depth: int) -> np.ndarray:
    pos = [0.0, 1.0]
    for _ in range(depth):
        new = []
        for i in range(len(pos) - 1):
            l, r = pos[i], pos[i + 1]
            new.append(l)
            new.append(l + (r - l) / 3.0)
        new.append(pos[-1])
        pos = new
    p32 = np.asarray(pos, dtype=np.float32)
    idx = (p32 * np.float32(seq_len - 1)).astype(np.int64)
    return np.unique(idx)


def _index_sets():
    sets = [_cantor_indices(L, d) for d in range(N_DEPTHS)]
    union = sets[-1]
    assert union[0] == 0 and len(union) == 129
    cols = union[union != 0]  # 128 non-j* indices, sorted
    member = np.zeros((N_DEPTHS, len(cols)), dtype=np.float32)
    for d, s in enumerate(sets):
        member[d] = np.isin(cols, s)
    return cols, member


_COLS, _MEMBER = _index_sets()

_NC_CACHE = None

_SPILL_SEQ = [0]


def _dedupe_ldweights(nc):
    """Delete a standalone InstLdweights whose weights AP is identical to
    the immediately preceding PE Ldweights (the stationary is already in the
    array; consecutive matmuls share it)."""
    for f in nc.m.functions:
        for bb in f.blocks:
            insts = bb.instructions
            last_ldw_ap = None
            idx = 0
            while idx < len(insts):
                inst = insts[idx]
                if str(inst.engine) != "EngineType.PE":
                    idx += 1
                    continue
                tn = type(inst).__name__
                if tn == "InstLdweights":
                    ap = str(inst.ins[0]) if inst.ins else None
                    si = inst.sync_info
                    has_sync = si is not None and (si.on_wait or si.on_update)
                    if ap is not None and ap == last_ldw_ap and not has_sync:
                        del insts[idx]
                        continue
                    last_ldw_ap = ap
                idx += 1


def _legalize_sync_commands(nc):
    """Walrus codegen caps sync commands (waits + updates) per ISA
    instruction at 2. Spill excess waits onto standalone EventSemaphore
    instructions inserted just before the offender on the same engine."""
    for f in nc.m.functions:
        for bb in f.blocks:
            insts = bb.instructions
            idx = 0
            while idx < len(insts):
                inst = insts[idx]
                si = inst.sync_info
                if si is None:
                    idx += 1
                    continue
                waits = list(si.on_wait or [])
                updates = list(si.on_update or [])
                assert len(updates) <= 2, (inst.name, updates)
                cap = 1 if isinstance(inst, mybir.InstDrain) else 2
                keep = max(0, cap - len(updates))
                if len(waits) <= keep:
                    idx += 1
                    continue
                spill, keep_waits = (
                    waits[: len(waits) - keep],
                    waits[len(waits) - keep :],
                )
                inst.sync_info = mybir.SyncInfo(on_wait=keep_waits, on_update=updates)
                pos = idx
                for i in range(0, len(spill), 2):
                    _SPILL_SEQ[0] += 1
                    ev = mybir.InstEventSemaphore(
                        name=f"WSPILL-{_SPILL_SEQ[0]}", ins=[], outs=[]
                    )
                    ev.engine = inst.engine
                    ev.sync_info = mybir.SyncInfo(
                        on_wait=spill[i : i + 2], on_update=[]
                    )
                    insts.insert(pos, ev)
                    pos += 1
                    idx += 1
                idx += 1


def _build_nc(nrep=1, mode="full", depth=3, style="c"):
    # mode: "full" | "dma" (skip compute) | "compute" (skip per-rep DMAs)
    nc = bass.Bass()
    qb = nc.declare_dram_parameter(
        "qb", [NBLK, 128, 8, BLK], F16, isOutput=False
    )
    # cpack[p, :]: kt (8*128) | mt (8) | vp (1024) packed along columns
    cpack = nc.declare_dram_parameter(
        "cpack", [128, 8 * 128 + N_DEPTHS + D], F16, isOutput=False
    )
    small = nc.declare_dram_parameter(
        "small", [N_DEPTHS, _SMALL_COLS], F16, isOutput=False
    )
    out = nc.declare_dram_parameter(
        "out", [128, NBLK * NTIL, D], F16, isOutput=True
    )

    npairs = (NBLK // 2) * nrep

    def pair_blocks(p):
        return (p * 2) // NBLK, (p * 2) % NBLK, (p * 2 + 1) % NBLK

    copy_eng = []
    for part in _KCOPY.split(","):
        copy_eng.append([c == "A" for c in part])

    with TileContext(nc) as tc:
        with (
            tc.tile_pool(name="const", bufs=1) as cpool,
            tc.tile_pool(name="qts", bufs=2) as qpool,
            tc.tile_pool(name="work", bufs=2) as wpool,
            tc.tile_pool(name="osb", bufs=4) as opool,
            tc.tile_pool(name="ps_st", bufs=2, space="PSUM") as ps_st,
            tc.tile_pool(name="ps_zc", bufs=2, space="PSUM") as ps_zc,
            tc.tile_pool(name="ps_o", bufs=2, space="PSUM") as ps_o,
        ):
            # ---- head prefetch on SP (+ACT for q0/q1 chunk split)
            cp_t = cpool.tile([128, 8 * 128 + N_DEPTHS + D], F16, tag="cpack")
            nc.sync.dma_start(out=cp_t[:, 0:1032], in_=cpack[:, 0:1032])
            sm_t = cpool.tile([N_DEPTHS, _SMALL_COLS], F16, tag="small")
            nc.sync.dma_start(out=sm_t, in_=small[:])

            qtiles = {}

            def load_q_chunks(blk, rep=0):
                q_b = qpool.tile([128, 8, BLK], F16, tag=f"qt_{blk}")
                for c in range(8):
                    eng = nc.scalar if c % 2 == 0 else nc.sync
                    eng.dma_start(out=q_b[:, c, :], in_=qb[blk, :, c, :])
                qtiles[(rep, blk)] = q_b
                return q_b

            def load_q_slab(blk, rep):
                q_b = qpool.tile([128, 8, BLK], F16, tag=f"qt_{blk}")
                nc.sync.dma_start(out=q_b, in_=qb[blk])
                qtiles[(rep, blk)] = q_b
                return q_b

            load_q_chunks(0)
            nc.sync.dma_start(out=cp_t[:, 1032:1544], in_=cpack[:, 1032:1544])
            load_q_chunks(1)
            nc.sync.dma_start(out=cp_t[:, 1544:], in_=cpack[:, 1544:])
            if mode == "compute":
                load_q_slab(2, 0)
                load_q_slab(3, 0)

            kt_t = [cp_t[:, c * 128 : (c + 1) * 128] for c in range(8)]
            mt_t = cp_t[:, 1024 : 1024 + N_DEPTHS]
            vp_t = cp_t[:, 1024 + N_DEPTHS : 1024 + N_DEPTHS + D]
            m8w_t = sm_t[:, 0:128]

            out_r = out.rearrange("p (b t) d -> p b t d", t=NTIL)

            if mode == "dma":
                for i in range(NBLK * nrep):
                    rep, blk = i // NBLK, i % NBLK
                    if (rep, blk) in qtiles:
                        q_b = qtiles.pop((rep, blk))
                    else:
                        q_b = load_q_slab(blk, rep)
                        qtiles.pop((rep, blk))
                    o_blk = opool.tile([128, NTIL, D], F16, tag="osb")
                    for t in range(NTIL):
                        nc.vector.tensor_copy(
                            o_blk[:, t, 0:512], q_b[:, 0, 0:512]
                        )
                        nc.vector.tensor_copy(
                            o_blk[:, t, 512:1024], q_b[:, 1, 0:512]
                        )
                    for half in range(2):
                        eng = nc.sync if blk == NBLK - 1 else nc.gpsimd
                        eng.dma_start(
                            out=out_r[:, blk, half * 2 : half * 2 + 2],
                            in_=o_blk[:, half * 2 : half * 2 + 2],
                        )

            state = {}  # p -> (eta, etb)
            rstate = {}  # p -> [rta, rtb]
            npairs_run = -2 if mode == "dma" else npairs
            last_rep = nrep - 1

            for p in range(npairs_run + 2):
                cur = p if p < npairs else None
                zp = p - 1 if 0 <= p - 1 < npairs else None
                dp = p - 2 if p - 2 >= 0 else None

                # phase 0: q prefetch for pair p+1, all on the SP ring
                if mode != "compute" and p + 1 < npairs:
                    rep1, ba1, bb1 = pair_blocks(p + 1)
                    for blk in (ba1, bb1):
                        if (rep1, blk) not in qtiles:
                            load_q_slab(blk, rep1)

                # phase 1: C matmuls + A=E*C muls for pair dp
                halves = []
                if dp is not None:
                    rta, rtb = rstate.pop(dp)
                    eta_d, etb_d = state.pop(dp)
                    rep_d, ba_d, bb_d = pair_blocks(dp)
                    cta = ps_zc.tile([128, BLK], F32, tag="zc")
                    nc.tensor.matmul(cta, lhsT=m8w_t, rhs=rta,
                                     start=True, stop=True)
                    ctb = ps_zc.tile([128, BLK], F32, tag="zc")
                    nc.tensor.matmul(ctb, lhsT=m8w_t, rhs=rtb,
                                     start=True, stop=True)
                    at_a = wpool.tile([128, BLK], F16, tag="at", bufs=2)
                    at_b = wpool.tile([128, BLK], F16, tag="at", bufs=2)
                    with nc.allow_low_precision(reason="attention probs fp16"):
                        nc.vector.tensor_mul(at_a, eta_d, cta)
                        nc.vector.tensor_mul(at_b, etb_d, ctb)
                    o_a = opool.tile([128, NTIL, D], F16, tag="osb")
                    o_b = opool.tile([128, NTIL, D], F16, tag="osb")
                    for bi, (blk, at, o_blk) in enumerate(
                        ((ba_d, at_a, o_a), (bb_d, at_b, o_b))
                    ):
                        for t in range(NTIL):
                            halves.append((rep_d, bi, blk, t, at, o_blk))

                def emit_z_half(et):
                    zt = ps_zc.tile([N_DEPTHS, BLK], F32, tag="zc")
                    mm = nc.tensor.matmul(zt, lhsT=mt_t, rhs=et,
                                          start=True, stop=True,
                                          skip_group_check=True)
                    ztmp = wpool.tile([N_DEPTHS, BLK], F32, tag="ztmp", bufs=2)
                    if _KZ == "act":
                        nc.scalar.add(ztmp, zt, 1.0)
                    else:
                        nc.vector.tensor_scalar_add(ztmp, zt, 1.0)
                    rt = wpool.tile([N_DEPTHS, BLK], F16, tag="rt", bufs=4)
                    with nc.allow_low_precision(reason="attention probs fp16"):
                        nc.vector.reciprocal(rt, ztmp)
                    return mm, rt

                if zp is not None:
                    eta_z, etb_z = state[zp]

                hseq = [0]

                def emit_half(prev_mm):
                    """One AV tile-half matmul; on the tile's second half,
                    also the PSUM->SBUF tile copy and any half-block drain."""
                    i = hseq[0]
                    hseq[0] += 1
                    rep_d, bi, blk, t, at, o_blk = halves[i // 2]
                    h = i % 2
                    if h == 0:
                        o_t = ps_o.tile([128, D], F32, tag="ops")
                        emit_half.o_t = o_t
                    else:
                        o_t = emit_half.o_t
                    sl = slice(t * 128, (t + 1) * 128)
                    mm = nc.tensor.matmul(
                        o_t[:, h * BLK : (h + 1) * BLK], lhsT=at[:, sl],
                        rhs=vp_t[:, h * BLK : (h + 1) * BLK],
                        start=True, stop=True, skip_group_check=True,
                    )
                    if prev_mm is not None:
                        add_dep_helper(mm.ins, prev_mm.ins, sync=False,
                                       reason="pipe interleave")
                    if h == 1:
                        with nc.allow_low_precision(reason="fp16 output"):
                            if copy_eng[bi][t]:
                                nc.scalar.copy(o_blk[:, t], o_t)
                            else:
                                nc.vector.tensor_copy(o_blk[:, t], o_t)
                        if t % 2 == 1 and not (
                            mode == "compute" and rep_d != nrep - 1
                        ):
                            if rep_d == last_rep and blk == NBLK - 1:
                                eng = nc.sync if t == 1 else nc.gpsimd
                            else:
                                eng = nc.gpsimd
                            eng.dma_start(
                                out=out_r[:, blk, t - 1 : t + 1],
                                in_=o_blk[:, t - 1 : t + 1],
                            )
                    return mm

                if cur is not None:
                    rep_c, ba_c, bb_c = pair_blocks(cur)
                    if mode == "compute":
                        qa, qvb = qtiles[(0, ba_c)], qtiles[(0, bb_c)]
                    else:
                        qa = qtiles.pop((rep_c, ba_c))
                        qvb = qtiles.pop((rep_c, bb_c))
                    sta = ps_st.tile([128, BLK], F32, tag="st")
                    stb = ps_st.tile([128, BLK], F32, tag="st")
                    if p == 0:
                        # head: block-sequential so the first ST matmuls
                        # stream behind q0's per-chunk DMA arrivals
                        for c in range(8):
                            nc.tensor.matmul(
                                sta, lhsT=kt_t[c], rhs=qa[:, c, :],
                                start=(c == 0), stop=(c == 7),
                            )
                        for c in range(8):
                            nc.tensor.matmul(
                                stb, lhsT=kt_t[c], rhs=qvb[:, c, :],
                                start=(c == 0), stop=(c == 7),
                            )
                    else:
                        prev_mm = None
                        for c in range(8):
                            ma = nc.tensor.matmul(
                                sta, lhsT=kt_t[c], rhs=qa[:, c, :],
                                start=(c == 0), stop=(c == 7),
                                skip_group_check=True,
                            )
                            if prev_mm is not None:
                                add_dep_helper(ma.ins, prev_mm.ins, sync=False,
                                               reason="pipe interleave")
                            mb = nc.tensor.matmul(
                                stb, lhsT=kt_t[c], rhs=qvb[:, c, :],
                                start=(c == 0), stop=(c == 7),
                                skip_group_check=True,
                            )
                            add_dep_helper(mb.ins, ma.ins, sync=False,
                                           reason="pipe interleave")
                            prev_mm = mb
                            if zp is not None and c == 1:
                                zmm, rta_z = emit_z_half(eta_z)
                                add_dep_helper(zmm.ins, prev_mm.ins,
                                               sync=False, reason="pipe")
                                prev_mm = zmm
                            if zp is not None and c == 5:
                                zmm, rtb_z = emit_z_half(etb_z)
                                add_dep_helper(zmm.ins, prev_mm.ins,
                                               sync=False, reason="pipe")
                                prev_mm = zmm
                            for _ in range(2):
                                if hseq[0] < 2 * len(halves):
                                    prev_mm = emit_half(prev_mm)
                        while hseq[0] < 2 * len(halves):
                            prev_mm = emit_half(prev_mm)
                    eta = wpool.tile([128, BLK], F16, tag="et", bufs=6)
                    nc.scalar.activation(
                        eta, sta, mybir.ActivationFunctionType.Exp,
                        scale=float(INV_SQRT_D),
                    )
                    etb = wpool.tile([128, BLK], F16, tag="et", bufs=6)
                    nc.scalar.activation(
                        etb, stb, mybir.ActivationFunctionType.Exp,
                        scale=float(INV_SQRT_D),
                    )
                    state[cur] = (eta, etb)
                else:
                    if zp is not None:
                        _, rta_z = emit_z_half(eta_z)
                        _, rtb_z = emit_z_half(etb_z)
                    prev_mm = None
                    while hseq[0] < 2 * len(halves):
                        prev_mm = emit_half(prev_mm)

                if zp is not None:
                    rstate[zp] = (rta_z, rtb_z)

    if style == "c":
        _dedupe_ldweights(nc)
    _legalize_sync_commands(nc)
    return nc


def _prepare_in_maps(query, key, value, scale_weights, scale_temperature):
    sw = np.asarray(scale_weights, dtype=np.float64)[:N_DEPTHS]
    temp = float(np.asarray(scale_temperature, dtype=np.float64))
    e = np.exp(sw / temp - np.max(sw / temp))
    w = (e / e.sum()).astype(np.float32)  # [8]

    mt = _MEMBER.T.astype(np.float16)  # [128, 8]
    m8w = (_MEMBER * w[:, None]).astype(np.float16)  # [8, 128]

    in_maps = []
    vstars = []
    for core in range(NCORES):
        b, half = core // 2, core % 2
        rows = slice(half * ROWS_PER_CORE, (half + 1) * ROWS_PER_CORE)
        q = np.ascontiguousarray(query[b, rows])  # [2048, D] f32
        k0 = key[b, 0].astype(np.float32)  # [D]
        k_u = key[b, _COLS] - k0[None, :]  # [128, D] f32, shifted
        vstar = value[b, 0].astype(np.float32)  # [D]
        vp = (value[b, _COLS] - vstar[None, :]).astype(np.float16)

        qt = q.T.astype(np.float16)  # [D, 2048]
        qb = np.ascontiguousarray(
            qt.reshape(8, 128, NBLK, BLK).transpose(2, 1, 0, 3)
        )
        ktp = np.ascontiguousarray(
            k_u.T.astype(np.float16).reshape(8, 128, 128).transpose(1, 0, 2)
        ).reshape(128, 1024)  # [p, c*128+j]
        cpack = np.concatenate([ktp, mt, vp], axis=1)  # [128, 2056]
        smallt = np.zeros((N_DEPTHS, _SMALL_COLS), dtype=np.float16)
        smallt[:, 0:128] = m8w
        in_maps.append(
            {
                "qb": qb,
                "cpack": np.ascontiguousarray(cpack),
                "small": smallt,
            }
        )
        vstars.append(vstar)
    return in_maps, vstars


def _unshard(results, vstars):
    outp = np.empty((B, L, D), dtype=np.float32)
    for core in range(NCORES):
        b, half = core // 2, core % 2
        rows = slice(half * ROWS_PER_CORE, (half + 1) * ROWS_PER_CORE)
        o = results[core]["out"]  # [128, 16, 1024] fp16
        o = o.transpose(1, 0, 2).reshape(ROWS_PER_CORE, D)
        outp[b, rows] = o.astype(np.float32) + vstars[core][None, :]
    return outp


def _run(query, key, value, t, scale_weights, scale_temperature, trace=False):
    global _NC_CACHE
    query = np.asarray(query, dtype=np.float32)
    key = np.asarray(key, dtype=np.float32)
    value = np.asarray(value, dtype=np.float32)
    assert query.shape == (B, L, D)

    in_maps, vstars = _prepare_in_maps(
        query, key, value, scale_weights, scale_temperature
    )
    if _NC_CACHE is None:
        _NC_CACHE = _build_nc()
    res = run_bass_kernel_spmd(
        _NC_CACHE, in_maps, core_ids=list(range(NCORES)), trace=trace
    )
    return _unshard(res.results, vstars), res


def kernel(query, key, value, t, scale_weights, scale_temperature):
    out, _ = _run(query, key, value, t, scale_weights, scale_temperature, trace=False)
    return out
